# revision 15
# baseline (speedup 1.0000x reference)
"""Trainium2 Bass kernel for nn_BatchEdgeGenerator (batched cosine-sim KNN).

Math (per batch b):
    sim[a, n] = <x_act[:, a], x_sens[:, n]> / (|x_act[:, a]| * |x_sens[:, n]|)
    idx[a, :] = top-16 indices of sim[a, :]**2     (descending)
    val[a, :] = sim[a, idx]

Sharding: pure data parallel — 32 batches / 8 cores = 4 batches per core.
No collectives. Each core returns val[4, 256, 16] f32 and idx[4, 256, 16] u32;
the host reassembles edges/weights (the source-node row of `edges` is a
data-independent constant).

Device algorithm per batch (PE-roofline aware: fp32 matmul streams at
~19.6 TF/s = its fp32 peak, so the 256x4096x512 sim matmul is a ~55us/batch
floor — every other engine must stay under that):
  - sim = x_act^T @ x_sens on PE, S=4096 contracted in 32 chunks of 128
    partitions, both 128-row output chunks accumulated into one [128,1024]
    PSUM tile (two banks). The first and last group's input DMAs are split
    into 2-chunk sub-tiles: the first so the PE starts early, the last so
    its squares land early.
  - The last group's matmuls run all-chunk0 then all-chunk1, with the norm
    reduction + rs pipeline + chunk0's ENTIRE top-k emitted between, so the
    serial top-k tail shrinks to one chunk.
  - Norms: squares on ACT (wide tiles), folded per group/part and
    accumulated into [128,512]/[128,256] running sums (sensors on DVE,
    actuators on GPSIMD), then reduced across partitions with a single
    all-ones [128,128] matmul that lands the column sums ALREADY BROADCAST
    across all 128 partitions of PSUM.
  - rs = sqrt(1/ns) computed directly in the broadcast layout (DVE
    reciprocal + ACT sqrt on [128,512]); the per-row factor 1/na does not
    change each row's top-k, so it is applied to the 16 selected values
    only (ra2 = 1/na, transposed to per-partition via K=1 matmuls, applied
    inside the ACT sqrt's scale).
  - Top-16 of simn^2 per row with the sign of simn stashed in the mantissa
    LSB: sqs = (simn^2 bits >>1<<1) | (simn > 0). Two rounds of the DVE
    max-8 / max-index-8 ops with a match_replace knockout give sorted
    top-16 values and indices without any gather.
"""

import numpy as np

B = 32          # total batches
NCORES = 8
BPC = B // NCORES
S = 4096
A = 256
N = 512
K = 16
P = 128
NCHUNK = S // P     # 32 contraction chunks
G = 8               # chunks per DMA group
NG = NCHUNK // G
SUB = 2             # chunks per sub-DMA in split groups
NSUB = G // SUB

_CACHE = {}


def _build_program():
    import concourse.tile as tile
    from concourse import bacc, mybir

    f32 = mybir.dt.float32
    u32 = mybir.dt.uint32
    Alu = mybir.AluOpType
    Act = mybir.ActivationFunctionType

    nc = bacc.Bacc(
        "TRN2",
        target_bir_lowering=False,
        debug=False,
        enable_asserts=False,
    )
    x_act = nc.dram_tensor("x_act", [BPC, S, A], f32, kind="ExternalInput")
    x_sens = nc.dram_tensor("x_sens", [BPC, S, N], f32, kind="ExternalInput")
    out_val = nc.dram_tensor("out_val", [BPC, A, K], f32, kind="ExternalOutput")
    out_idx = nc.dram_tensor("out_idx", [BPC, A, K], u32, kind="ExternalOutput")

    with tile.TileContext(nc) as tc:
        from contextlib import ExitStack

        with ExitStack() as ctx:
            singles = ctx.enter_context(tc.tile_pool(name="singles", bufs=1))
            in_pool = ctx.enter_context(tc.tile_pool(name="inp", bufs=2))
            in0_pool = ctx.enter_context(tc.tile_pool(name="inp0", bufs=NSUB))
            in3_pool = ctx.enter_context(tc.tile_pool(name="inp3", bufs=NSUB))
            sq_pool = ctx.enter_context(tc.tile_pool(name="sq", bufs=2))
            acc_pool = ctx.enter_context(tc.tile_pool(name="acc", bufs=2))
            topk_pool = ctx.enter_context(tc.tile_pool(name="topk", bufs=2))
            small_pool = ctx.enter_context(tc.tile_pool(name="small", bufs=2))
            out_pool = ctx.enter_context(tc.tile_pool(name="outp", bufs=4))
            ps_sim = ctx.enter_context(
                tc.tile_pool(name="ps_sim", bufs=2, space="PSUM")
            )
            ps_bc = ctx.enter_context(
                tc.tile_pool(name="ps_bc", bufs=1, space="PSUM")
            )
            ps_na = ctx.enter_context(
                tc.tile_pool(name="ps_na", bufs=1, space="PSUM")
            )

            # all-ones [128,128] fp32: one matmul both reduces over
            # partitions AND broadcasts the result to every output row
            ones_mat = singles.tile([P, P], f32)
            nc.vector.memset(ones_mat, 1.0)
            ones_11 = singles.tile([1, 1], f32)
            nc.vector.memset(ones_11, 1.0)

            def fold_tile(eng, t, parts, width):
                """Fold [P, parts*width] halves down to [P, width]."""
                w = parts // 2
                while w >= 1:
                    eng.tensor_add(
                        t[:, 0 : w * width],
                        t[:, 0 : w * width],
                        t[:, w * width : 2 * w * width],
                    )
                    w //= 2

            def load_group_whole(b, g):
                act_g = in_pool.tile([P, G, A], f32, tag="act")
                sens_g = in_pool.tile([P, G, N], f32, tag="sens")
                r0, r1 = g * G * P, (g + 1) * G * P
                nc.sync.dma_start(
                    out=act_g,
                    in_=x_act[b, r0:r1, :].rearrange("(c p) a -> p c a", p=P),
                )
                nc.sync.dma_start(
                    out=sens_g,
                    in_=x_sens[b, r0:r1, :].rearrange("(c p) n -> p c n", p=P),
                )

                def squares(acc_s, acc_a, first):
                    H = G // 2
                    for h in range(2):
                        sq_s = sq_pool.tile([P, H * N], f32, tag="sq_s")
                        sq_a = sq_pool.tile([P, H * A], f32, tag="sq_a")
                        nc.scalar.activation(
                            sq_s,
                            sens_g[:, h * H : (h + 1) * H, :].rearrange(
                                "p c n -> p (c n)"
                            ),
                            Act.Square,
                        )
                        nc.scalar.activation(
                            sq_a,
                            act_g[:, h * H : (h + 1) * H, :].rearrange(
                                "p c a -> p (c a)"
                            ),
                            Act.Square,
                        )
                        fold_tile(nc.vector, sq_s, H, N)
                        fold_tile(nc.gpsimd, sq_a, H, A)
                        if first and h == 0:
                            nc.vector.tensor_copy(acc_s, sq_s[:, 0:N])
                            nc.gpsimd.tensor_copy(acc_a, sq_a[:, 0:A])
                        else:
                            nc.vector.tensor_add(acc_s, acc_s, sq_s[:, 0:N])
                            nc.gpsimd.tensor_add(acc_a, acc_a, sq_a[:, 0:A])

                return (
                    lambda c: act_g[:, c, :],
                    lambda c: sens_g[:, c, :],
                    squares,
                )

            def load_group_split(b, g, pool):
                act_parts, sens_parts = [], []
                for h in range(NSUB):
                    ap = pool.tile([P, SUB, A], f32, tag="actp")
                    sp = pool.tile([P, SUB, N], f32, tag="sensp")
                    r0 = (g * G + h * SUB) * P
                    r1 = (g * G + (h + 1) * SUB) * P
                    nc.sync.dma_start(
                        out=ap,
                        in_=x_act[b, r0:r1, :].rearrange("(c p) a -> p c a", p=P),
                    )
                    nc.sync.dma_start(
                        out=sp,
                        in_=x_sens[b, r0:r1, :].rearrange("(c p) n -> p c n", p=P),
                    )
                    act_parts.append(ap)
                    sens_parts.append(sp)

                def squares(acc_s, acc_a, first):
                    for h in range(NSUB):
                        sq_s = sq_pool.tile([P, SUB * N], f32, tag="sq_s0")
                        sq_a = sq_pool.tile([P, SUB * A], f32, tag="sq_a0")
                        nc.scalar.activation(
                            sq_s,
                            sens_parts[h].rearrange("p c n -> p (c n)"),
                            Act.Square,
                        )
                        nc.scalar.activation(
                            sq_a,
                            act_parts[h].rearrange("p c a -> p (c a)"),
                            Act.Square,
                        )
                        fold_tile(nc.vector, sq_s, SUB, N)
                        fold_tile(nc.gpsimd, sq_a, SUB, A)
                        if first and h == 0:
                            nc.vector.tensor_copy(acc_s, sq_s[:, 0:N])
                            nc.gpsimd.tensor_copy(acc_a, sq_a[:, 0:A])
                        else:
                            nc.vector.tensor_add(acc_s, acc_s, sq_s[:, 0:N])
                            nc.gpsimd.tensor_add(acc_a, acc_a, sq_a[:, 0:A])

                return (
                    lambda c: act_parts[c // SUB][:, c % SUB, :],
                    lambda c: sens_parts[c // SUB][:, c % SUB, :],
                    squares,
                )

            def emit_tail(b, sim_ps, acc_s, acc_a, deferred):
                """Norms -> rs pipeline -> per-chunk top-16 -> outputs.

                For the last batch `deferred` holds the last group's
                accessors and the chunk1 matmuls are emitted between
                chunk0's and chunk1's top-k so the serial tail is short.
                """
                bc_ps = ps_bc.tile([P, N], f32, tag="bc")
                nc.tensor.matmul(bc_ps, ones_mat, acc_s, start=True, stop=True)
                inv_ns = small_pool.tile([P, N], f32, tag="invns")
                nc.vector.reciprocal(inv_ns, bc_ps)
                rs_bc = small_pool.tile([P, N], f32, tag="rsbc")
                nc.scalar.activation(rs_bc, inv_ns, Act.Sqrt)

                na_ps = ps_na.tile([1, A], f32, tag="na")
                nc.tensor.matmul(
                    na_ps, ones_mat[:, 0:1], acc_a, start=True, stop=True
                )
                na_row = small_pool.tile([1, A], f32, tag="narow")
                nc.scalar.copy(na_row, na_ps)
                ra_ps = ps_na.tile([P, 2], f32, tag="ra")
                nc.tensor.matmul(
                    ra_ps[:, 0:1], na_row[:, 0:P], ones_11, start=True, stop=True
                )
                nc.tensor.matmul(
                    ra_ps[:, 1:2], na_row[:, P:A], ones_11, start=True, stop=True
                )
                ra2_sb = small_pool.tile([P, 2], f32, tag="ra2sb")
                nc.vector.reciprocal(ra2_sb, ra_ps)

                m16 = out_pool.tile([P, 2, K], f32, tag="m16")
                idx16 = out_pool.tile([P, 2, K], u32, tag="idx16")

                def topk_chunk(c2):
                    simn = topk_pool.tile([P, N], f32, tag="simn")
                    nc.vector.tensor_mul(simn, sim_ps[:, c2, :], rs_bc)
                    sq_t = topk_pool.tile([P, N], f32, tag="sqt")
                    nc.scalar.activation(sq_t, simn, Act.Square)
                    sqc_t = topk_pool.tile([P, N], u32, tag="sqct")
                    nc.vector.tensor_scalar(
                        sqc_t,
                        sq_t.bitcast(u32),
                        1,
                        1,
                        op0=Alu.logical_shift_right,
                        op1=Alu.logical_shift_left,
                    )
                    pos_u = topk_pool.tile([P, N], u32, tag="posu")
                    nc.vector.tensor_scalar(
                        pos_u, simn, 0.0, None, op0=Alu.is_gt
                    )
                    sqs_t = topk_pool.tile([P, N], f32, tag="sqst")
                    nc.vector.tensor_tensor(
                        out=sqs_t.bitcast(u32),
                        in0=sqc_t,
                        in1=pos_u,
                        op=Alu.bitwise_or,
                    )
                    nc.vector.max(m16[:, c2, 0:8], sqs_t)
                    nc.vector.max_index(
                        idx16[:, c2, 0:8], m16[:, c2, 0:8], sqs_t
                    )
                    sqs2_t = topk_pool.tile([P, N], f32, tag="sqs2")
                    nc.vector.match_replace(
                        sqs2_t, m16[:, c2, 0:8], sqs_t, -1.0
                    )
                    nc.vector.max(m16[:, c2, 8:16], sqs2_t)
                    nc.vector.max_index(
                        idx16[:, c2, 8:16], m16[:, c2, 8:16], sqs2_t
                    )

                def recon_chunk(c2):
                    sgn_u = out_pool.tile([P, K], u32, tag="sgn")
                    nc.vector.tensor_scalar(
                        sgn_u,
                        m16[:, c2, :].bitcast(u32),
                        1,
                        None,
                        op0=Alu.bitwise_and,
                    )
                    vabs = out_pool.tile([P, K], f32, tag="vabs")
                    nc.scalar.activation(
                        vabs,
                        m16[:, c2, :],
                        Act.Sqrt,
                        scale=ra2_sb[:, c2 : c2 + 1],
                    )
                    nvabs = out_pool.tile([P, K], f32, tag="nvabs")
                    nc.vector.tensor_scalar_mul(nvabs, vabs, -1.0)
                    v16 = out_pool.tile([P, K], f32, tag="v16")
                    nc.vector.select(v16, sgn_u, vabs, nvabs)
                    nc.sync.dma_start(
                        out=out_val[b, c2 * P : (c2 + 1) * P, :], in_=v16
                    )
                    nc.sync.dma_start(
                        out=out_idx[b, c2 * P : (c2 + 1) * P, :],
                        in_=idx16[:, c2, :],
                    )

                topk_chunk(0)
                recon_chunk(0)
                if deferred is not None:
                    act_c, sens_c = deferred
                    g = NG - 1
                    for c in range(G):
                        k = g * G + c
                        nc.tensor.matmul(
                            sim_ps[:, 1, :],
                            act_c(c)[:, P:A],
                            sens_c(c),
                            start=False,
                            stop=k == NCHUNK - 1,
                        )
                topk_chunk(1)
                recon_chunk(1)

            pending = None
            for b in range(BPC):
                last_batch = b == BPC - 1
                sim_ps = ps_sim.tile([P, 2, N], f32, tag="sim")  # 2 banks
                acc_s = acc_pool.tile([P, N], f32, tag="acc_s")
                acc_a = acc_pool.tile([P, A], f32, tag="acc_a")

                deferred = None
                for g in range(NG):
                    defer_c1 = last_batch and g == NG - 1
                    if b == 0 and g == 0:
                        act_c, sens_c, squares = load_group_split(b, g, in0_pool)
                    elif defer_c1:
                        act_c, sens_c, squares = load_group_split(b, g, in3_pool)
                    else:
                        act_c, sens_c, squares = load_group_whole(b, g)
                    squares(acc_s, acc_a, g == 0)
                    for c in range(G):
                        k = g * G + c
                        nc.tensor.matmul(
                            sim_ps[:, 0, :],
                            act_c(c)[:, 0:P],
                            sens_c(c),
                            start=k == 0,
                            stop=k == NCHUNK - 1,
                        )
                        if not defer_c1:
                            nc.tensor.matmul(
                                sim_ps[:, 1, :],
                                act_c(c)[:, P:A],
                                sens_c(c),
                                start=k == 0,
                                stop=k == NCHUNK - 1,
                            )
                    if defer_c1:
                        deferred = (act_c, sens_c)
                    if g == 0 and pending is not None:
                        pending()
                        pending = None

                if last_batch:
                    emit_tail(b, sim_ps, acc_s, acc_a, deferred)
                else:
                    def pending(
                        b=b, sim_ps=sim_ps, acc_s=acc_s, acc_a=acc_a
                    ):
                        emit_tail(b, sim_ps, acc_s, acc_a, None)

    nc.compile()
    return nc


def _get_program():
    if "nc" not in _CACHE:
        _CACHE["nc"] = _build_program()
    return _CACHE["nc"]


def _run(x_actuators, x_sensors, trace=False):
    from concourse.bass_utils import run_bass_kernel_spmd

    nc = _get_program()
    xa = np.ascontiguousarray(np.asarray(x_actuators, dtype=np.float32)).reshape(
        NCORES, BPC, S, A
    )
    xs = np.ascontiguousarray(np.asarray(x_sensors, dtype=np.float32)).reshape(
        NCORES, BPC, S, N
    )
    in_maps = [{"x_act": xa[i], "x_sens": xs[i]} for i in range(NCORES)]
    res = run_bass_kernel_spmd(
        nc, in_maps, list(range(NCORES)), trace=trace
    )
    vals = np.concatenate([r["out_val"] for r in res.results], axis=0)  # (B,A,K)
    idxs = np.concatenate([r["out_idx"] for r in res.results], axis=0)  # (B,A,K)

    weights = vals.reshape(B, A * K).astype(np.float32)
    src = np.tile(
        np.repeat(np.arange(A, dtype=np.int32), K)[None, :], (B, 1)
    )
    tgt = idxs.reshape(B, A * K).astype(np.int32)
    edges = np.stack([src, tgt], axis=1)
    return (edges, weights), res


def kernel(x_actuators, x_sensors):
    (edges, weights), _ = _run(x_actuators, x_sensors, trace=False)
    return edges, weights


def kernel_traced(x_actuators, x_sensors):
    """Like kernel() but returns ((edges, weights), BassKernelResults)."""
    return _run(x_actuators, x_sensors, trace=True)


# revision 17
# speedup vs baseline: 1.0559x; 1.0559x over previous
"""Trainium2 Bass kernel for nn_BatchEdgeGenerator (batched cosine-sim KNN).

Math (per batch b):
    sim[a, n] = <x_act[:, a], x_sens[:, n]> / (|x_act[:, a]| * |x_sens[:, n]|)
    idx[a, :] = top-16 indices of sim[a, :]**2     (descending)
    val[a, :] = sim[a, idx]

Sharding: pure data parallel — 32 batches / 8 cores = 4 batches per core.
No collectives. Each core returns val[4, 256, 16] f32 and idx[4, 256, 16] u32;
the host reassembles edges/weights (the source-node row of `edges` is a
data-independent constant).

Device algorithm per batch (PE-roofline aware: fp32 matmul streams at
~19.6 TF/s = its fp32 peak, so the 256x4096x512 sim matmul is a ~55us/batch
floor — every other engine must stay under that):
  - sim = x_act^T @ x_sens on PE, S=4096 contracted in 32 chunks of 128
    partitions, both 128-row output chunks accumulated into one [128,1024]
    PSUM tile (two banks). The first and last group's input DMAs are split
    into 2-chunk sub-tiles: the first so the PE starts early, the last so
    its squares land early.
  - The last group's matmuls run all-chunk0 then all-chunk1, with the norm
    reduction + rs pipeline + chunk0's ENTIRE top-k emitted between, so the
    serial top-k tail shrinks to one chunk.
  - Norms: squares on ACT (wide tiles), folded per group/part and
    accumulated into [128,512]/[128,256] running sums (sensors on DVE,
    actuators on GPSIMD), then reduced across partitions with a single
    all-ones [128,128] matmul that lands the column sums ALREADY BROADCAST
    across all 128 partitions of PSUM.
  - rs = sqrt(1/ns) computed directly in the broadcast layout (DVE
    reciprocal + ACT sqrt on [128,512]); the per-row factor 1/na does not
    change each row's top-k, so it is applied to the 16 selected values
    only (ra2 = 1/na, transposed to per-partition via K=1 matmuls, applied
    inside the ACT sqrt's scale).
  - Top-16 of simn^2 per row with the sign of simn stashed in the mantissa
    LSB: sqs = (simn^2 bits >>1<<1) | (simn > 0). Two rounds of the DVE
    max-8 / max-index-8 ops with a match_replace knockout give sorted
    top-16 values and indices without any gather.
"""

import numpy as np

B = 32          # total batches
NCORES = 8
BPC = B // NCORES
S = 4096
A = 256
N = 512
K = 16
P = 128
NCHUNK = S // P     # 32 contraction chunks
G = 8               # chunks per DMA group
NG = NCHUNK // G
SUB = 2             # chunks per sub-DMA in split groups
NSUB = G // SUB

_CACHE = {}


def _build_program():
    import concourse.tile as tile
    from concourse import bacc, mybir

    f32 = mybir.dt.float32
    u32 = mybir.dt.uint32
    Alu = mybir.AluOpType
    Act = mybir.ActivationFunctionType

    nc = bacc.Bacc(
        "TRN2",
        target_bir_lowering=False,
        debug=False,
        enable_asserts=False,
    )
    x_act = nc.dram_tensor("x_act", [BPC, S, A], f32, kind="ExternalInput")
    x_sens = nc.dram_tensor("x_sens", [BPC, S, N], f32, kind="ExternalInput")
    out_val = nc.dram_tensor("out_val", [BPC, A, K], f32, kind="ExternalOutput")
    out_idx = nc.dram_tensor("out_idx", [BPC, A, K], u32, kind="ExternalOutput")

    with tile.TileContext(nc) as tc:
        from contextlib import ExitStack

        with ExitStack() as ctx:
            singles = ctx.enter_context(tc.tile_pool(name="singles", bufs=1))
            in_pool = ctx.enter_context(tc.tile_pool(name="inp", bufs=2))
            in0_pool = ctx.enter_context(tc.tile_pool(name="inp0", bufs=NSUB))
            in3_pool = ctx.enter_context(tc.tile_pool(name="inp3", bufs=NSUB))
            sq_pool = ctx.enter_context(tc.tile_pool(name="sq", bufs=2))
            acc_pool = ctx.enter_context(tc.tile_pool(name="acc", bufs=2))
            topk_pool = ctx.enter_context(tc.tile_pool(name="topk", bufs=2))
            small_pool = ctx.enter_context(tc.tile_pool(name="small", bufs=2))
            out_pool = ctx.enter_context(tc.tile_pool(name="outp", bufs=4))
            ps_sim = ctx.enter_context(
                tc.tile_pool(name="ps_sim", bufs=2, space="PSUM")
            )
            ps_bc = ctx.enter_context(
                tc.tile_pool(name="ps_bc", bufs=1, space="PSUM")
            )
            ps_na = ctx.enter_context(
                tc.tile_pool(name="ps_na", bufs=1, space="PSUM")
            )

            # all-ones [128,128] fp32: one matmul both reduces over
            # partitions AND broadcasts the result to every output row
            ones_mat = singles.tile([P, P], f32)
            nc.vector.memset(ones_mat, 1.0)
            ones_11 = singles.tile([1, 1], f32)
            nc.vector.memset(ones_11, 1.0)

            def fold_tile(eng, t, parts, width):
                """Fold [P, parts*width] halves down to [P, width]."""
                w = parts // 2
                while w >= 1:
                    eng.tensor_add(
                        t[:, 0 : w * width],
                        t[:, 0 : w * width],
                        t[:, w * width : 2 * w * width],
                    )
                    w //= 2

            def load_group_whole(b, g):
                act_g = in_pool.tile([P, G, A], f32, tag="act")
                sens_g = in_pool.tile([P, G, N], f32, tag="sens")
                r0, r1 = g * G * P, (g + 1) * G * P
                nc.sync.dma_start(
                    out=act_g,
                    in_=x_act[b, r0:r1, :].rearrange("(c p) a -> p c a", p=P),
                )
                nc.sync.dma_start(
                    out=sens_g,
                    in_=x_sens[b, r0:r1, :].rearrange("(c p) n -> p c n", p=P),
                )

                def squares(acc_s, acc_a, first):
                    H = G // 2
                    for h in range(2):
                        sq_s = sq_pool.tile([P, H * N], f32, tag="sq_s")
                        sq_a = sq_pool.tile([P, H * A], f32, tag="sq_a")
                        nc.scalar.activation(
                            sq_s,
                            sens_g[:, h * H : (h + 1) * H, :].rearrange(
                                "p c n -> p (c n)"
                            ),
                            Act.Square,
                        )
                        nc.scalar.activation(
                            sq_a,
                            act_g[:, h * H : (h + 1) * H, :].rearrange(
                                "p c a -> p (c a)"
                            ),
                            Act.Square,
                        )
                        fold_tile(nc.vector, sq_s, H, N)
                        fold_tile(nc.gpsimd, sq_a, H, A)
                        if first and h == 0:
                            nc.vector.tensor_copy(acc_s, sq_s[:, 0:N])
                            nc.gpsimd.tensor_copy(acc_a, sq_a[:, 0:A])
                        else:
                            nc.vector.tensor_add(acc_s, acc_s, sq_s[:, 0:N])
                            nc.gpsimd.tensor_add(acc_a, acc_a, sq_a[:, 0:A])

                return (
                    lambda c: act_g[:, c, :],
                    lambda c: sens_g[:, c, :],
                    squares,
                )

            def load_group_split(b, g, pool):
                act_parts, sens_parts = [], []
                for h in range(NSUB):
                    ap = pool.tile([P, SUB, A], f32, tag="actp")
                    sp = pool.tile([P, SUB, N], f32, tag="sensp")
                    r0 = (g * G + h * SUB) * P
                    r1 = (g * G + (h + 1) * SUB) * P
                    nc.sync.dma_start(
                        out=ap,
                        in_=x_act[b, r0:r1, :].rearrange("(c p) a -> p c a", p=P),
                    )
                    nc.sync.dma_start(
                        out=sp,
                        in_=x_sens[b, r0:r1, :].rearrange("(c p) n -> p c n", p=P),
                    )
                    act_parts.append(ap)
                    sens_parts.append(sp)

                def squares(acc_s, acc_a, first):
                    for h in range(NSUB):
                        sq_s = sq_pool.tile([P, SUB * N], f32, tag="sq_s0")
                        sq_a = sq_pool.tile([P, SUB * A], f32, tag="sq_a0")
                        nc.scalar.activation(
                            sq_s,
                            sens_parts[h].rearrange("p c n -> p (c n)"),
                            Act.Square,
                        )
                        nc.scalar.activation(
                            sq_a,
                            act_parts[h].rearrange("p c a -> p (c a)"),
                            Act.Square,
                        )
                        fold_tile(nc.vector, sq_s, SUB, N)
                        fold_tile(nc.gpsimd, sq_a, SUB, A)
                        if first and h == 0:
                            nc.vector.tensor_copy(acc_s, sq_s[:, 0:N])
                            nc.gpsimd.tensor_copy(acc_a, sq_a[:, 0:A])
                        else:
                            nc.vector.tensor_add(acc_s, acc_s, sq_s[:, 0:N])
                            nc.gpsimd.tensor_add(acc_a, acc_a, sq_a[:, 0:A])

                return (
                    lambda c: act_parts[c // SUB][:, c % SUB, :],
                    lambda c: sens_parts[c // SUB][:, c % SUB, :],
                    squares,
                )

            def emit_tail(b, sim_ps, acc_s, acc_a, deferred):
                """Norms -> rs pipeline -> per-chunk top-16 -> outputs.

                For the last batch `deferred` holds the last group's
                accessors and the chunk1 matmuls are emitted between
                chunk0's and chunk1's top-k so the serial tail is short.
                """
                bc_ps = ps_bc.tile([P, N], f32, tag="bc")
                nc.tensor.matmul(bc_ps, ones_mat, acc_s, start=True, stop=True)
                inv_ns = small_pool.tile([P, N], f32, tag="invns")
                nc.vector.reciprocal(inv_ns, bc_ps)
                rs_bc = small_pool.tile([P, N], f32, tag="rsbc")
                nc.scalar.activation(rs_bc, inv_ns, Act.Sqrt)

                na_ps = ps_na.tile([1, A], f32, tag="na")
                nc.tensor.matmul(
                    na_ps, ones_mat[:, 0:1], acc_a, start=True, stop=True
                )
                na_row = small_pool.tile([1, A], f32, tag="narow")
                nc.scalar.copy(na_row, na_ps)
                ra_ps = ps_na.tile([P, 2], f32, tag="ra")
                nc.tensor.matmul(
                    ra_ps[:, 0:1], na_row[:, 0:P], ones_11, start=True, stop=True
                )
                nc.tensor.matmul(
                    ra_ps[:, 1:2], na_row[:, P:A], ones_11, start=True, stop=True
                )
                ra2_sb = small_pool.tile([P, 2], f32, tag="ra2sb")
                nc.vector.reciprocal(ra2_sb, ra_ps)

                m16 = out_pool.tile([P, 2, K], f32, tag="m16")
                idx16 = out_pool.tile([P, 2, K], u32, tag="idx16")

                def topk_chunk(c2):
                    simn = topk_pool.tile([P, N], f32, tag="simn")
                    nc.vector.tensor_mul(simn, sim_ps[:, c2, :], rs_bc)
                    sq_t = topk_pool.tile([P, N], f32, tag="sqt")
                    nc.scalar.activation(sq_t, simn, Act.Square)
                    sqc_t = topk_pool.tile([P, N], u32, tag="sqct")
                    nc.vector.tensor_scalar(
                        sqc_t,
                        sq_t.bitcast(u32),
                        1,
                        1,
                        op0=Alu.logical_shift_right,
                        op1=Alu.logical_shift_left,
                    )
                    pos_u = topk_pool.tile([P, N], u32, tag="posu")
                    nc.vector.tensor_scalar(
                        pos_u, simn, 0.0, None, op0=Alu.is_gt
                    )
                    sqs_t = topk_pool.tile([P, N], f32, tag="sqst")
                    nc.vector.tensor_tensor(
                        out=sqs_t.bitcast(u32),
                        in0=sqc_t,
                        in1=pos_u,
                        op=Alu.bitwise_or,
                    )
                    nc.vector.max(m16[:, c2, 0:8], sqs_t)
                    nc.vector.max_index(
                        idx16[:, c2, 0:8], m16[:, c2, 0:8], sqs_t
                    )
                    sqs2_t = topk_pool.tile([P, N], f32, tag="sqs2")
                    nc.vector.match_replace(
                        sqs2_t, m16[:, c2, 0:8], sqs_t, -1.0
                    )
                    nc.vector.max(m16[:, c2, 8:16], sqs2_t)
                    nc.vector.max_index(
                        idx16[:, c2, 8:16], m16[:, c2, 8:16], sqs2_t
                    )

                def recon_chunk(c2):
                    sgn_u = out_pool.tile([P, K], u32, tag="sgn")
                    nc.vector.tensor_scalar(
                        sgn_u,
                        m16[:, c2, :].bitcast(u32),
                        1,
                        None,
                        op0=Alu.bitwise_and,
                    )
                    vabs = out_pool.tile([P, K], f32, tag="vabs")
                    nc.scalar.activation(
                        vabs,
                        m16[:, c2, :],
                        Act.Sqrt,
                        scale=ra2_sb[:, c2 : c2 + 1],
                    )
                    nvabs = out_pool.tile([P, K], f32, tag="nvabs")
                    nc.vector.tensor_scalar_mul(nvabs, vabs, -1.0)
                    v16 = out_pool.tile([P, K], f32, tag="v16")
                    nc.vector.select(v16, sgn_u, vabs, nvabs)
                    nc.scalar.dma_start(
                        out=out_val[b, c2 * P : (c2 + 1) * P, :], in_=v16
                    )
                    nc.scalar.dma_start(
                        out=out_idx[b, c2 * P : (c2 + 1) * P, :],
                        in_=idx16[:, c2, :],
                    )

                topk_chunk(0)
                recon_chunk(0)
                if deferred is not None:
                    act_c, sens_c = deferred
                    g = NG - 1
                    for c in range(G):
                        k = g * G + c
                        nc.tensor.matmul(
                            sim_ps[:, 1, :],
                            act_c(c)[:, P:A],
                            sens_c(c),
                            start=False,
                            stop=k == NCHUNK - 1,
                        )
                topk_chunk(1)
                recon_chunk(1)

            pending = None
            for b in range(BPC):
                last_batch = b == BPC - 1
                sim_ps = ps_sim.tile([P, 2, N], f32, tag="sim")  # 2 banks
                acc_s = acc_pool.tile([P, N], f32, tag="acc_s")
                acc_a = acc_pool.tile([P, A], f32, tag="acc_a")

                deferred = None
                for g in range(NG):
                    defer_c1 = last_batch and g == NG - 1
                    if b == 0 and g == 0:
                        act_c, sens_c, squares = load_group_split(b, g, in0_pool)
                    elif defer_c1:
                        act_c, sens_c, squares = load_group_split(b, g, in3_pool)
                    else:
                        act_c, sens_c, squares = load_group_whole(b, g)
                    squares(acc_s, acc_a, g == 0)
                    for c in range(G):
                        k = g * G + c
                        nc.tensor.matmul(
                            sim_ps[:, 0, :],
                            act_c(c)[:, 0:P],
                            sens_c(c),
                            start=k == 0,
                            stop=k == NCHUNK - 1,
                        )
                        if not defer_c1:
                            nc.tensor.matmul(
                                sim_ps[:, 1, :],
                                act_c(c)[:, P:A],
                                sens_c(c),
                                start=k == 0,
                                stop=k == NCHUNK - 1,
                            )
                    if defer_c1:
                        deferred = (act_c, sens_c)
                    if g == 0 and pending is not None:
                        pending()
                        pending = None

                if last_batch:
                    emit_tail(b, sim_ps, acc_s, acc_a, deferred)
                else:
                    def pending(
                        b=b, sim_ps=sim_ps, acc_s=acc_s, acc_a=acc_a
                    ):
                        emit_tail(b, sim_ps, acc_s, acc_a, None)

    nc.compile()
    return nc


def _get_program():
    if "nc" not in _CACHE:
        _CACHE["nc"] = _build_program()
    return _CACHE["nc"]


def _run(x_actuators, x_sensors, trace=False):
    from concourse.bass_utils import run_bass_kernel_spmd

    nc = _get_program()
    xa = np.ascontiguousarray(np.asarray(x_actuators, dtype=np.float32)).reshape(
        NCORES, BPC, S, A
    )
    xs = np.ascontiguousarray(np.asarray(x_sensors, dtype=np.float32)).reshape(
        NCORES, BPC, S, N
    )
    in_maps = [{"x_act": xa[i], "x_sens": xs[i]} for i in range(NCORES)]
    res = run_bass_kernel_spmd(
        nc, in_maps, list(range(NCORES)), trace=trace
    )
    vals = np.concatenate([r["out_val"] for r in res.results], axis=0)  # (B,A,K)
    idxs = np.concatenate([r["out_idx"] for r in res.results], axis=0)  # (B,A,K)

    weights = vals.reshape(B, A * K).astype(np.float32)
    src = np.tile(
        np.repeat(np.arange(A, dtype=np.int32), K)[None, :], (B, 1)
    )
    tgt = idxs.reshape(B, A * K).astype(np.int32)
    edges = np.stack([src, tgt], axis=1)
    return (edges, weights), res


def kernel(x_actuators, x_sensors):
    (edges, weights), _ = _run(x_actuators, x_sensors, trace=False)
    return edges, weights


def kernel_traced(x_actuators, x_sensors):
    """Like kernel() but returns ((edges, weights), BassKernelResults)."""
    return _run(x_actuators, x_sensors, trace=True)
